# revision 39
# baseline (speedup 1.0000x reference)
"""Trainium2 Bass kernel for nn_HamiltonianVersorNN.

Math: the reference energy reads only blade-0 of the final layer, and the
versor gate h*sigmoid(h[...,0:1]) makes blade-0 evolve as elementwise SiLU.
Backprop therefore collapses exactly to a 2-layer SiLU MLP on blade-0:

    a1 = A x + c1            A  = W1 @ W_in[:, ::32].T          [32, 6]
    a2 = W2 silu(a1) + c2    c1 = W1 @ b_in[::32] + b1[:, 0]
    dx = A.T (W2.T (w3 * silu'(a2)) * silu'(a1))
    out = x + dt * [dx[3:6], -dx[0:3]]

Sharding: pure data parallel over B*S*N positions, 8 cores, 16384
positions/core. On-chip layout packs 4 tokens per 128-partition column
(partition 32*tl + c holds channel c of token 4g+tl) so the W2 matmuls
contract over the full 128 partitions via block-diagonal stationaries.

Perf design (92.9us fp32 baseline -> ~40.9us):
- All matmuls at 1 cycle/row: x-consuming passes stream as float32r
  (TF32-like; 1 cyc/row at >=256 cols vs 4 for fp32), weight stationaries
  and activation movings are bf16.
- Biases ride matmuls/ones-rows: c1 and (W2 c1/2 + c2) sit on a constant
  ones-row appended to x in HBM (row 24), so no rank-1 bias matmuls and
  no Activation bias operands are needed.
- a2 = W2 silu(a1) + c2 is split as (W2/2)a1 + (W2/2)(a1 tanh(a1/2)):
  the first term composes with the input layer ((W2 A/2) x) and streams
  straight from x with no elementwise dependency; the only layer-1
  elementwise product is w = a1*tau1 on the DVE.
- Elementwise floor: ACT does Tanh(a1/2), dSilu(a1), dSilu(a2) (one act
  table, zero switches); DVE does the three PSUM-evacuating ops (w, g1,
  residual out=x+po), all bf16 outputs.
- Pipelining: 1024-col blocks with the back half (d2/l3/g1/l4/res/DMA)
  software-pipelined one block behind the front half; d2(k-1) is issued
  right after tau1(k) so it takes the second ACT slot of each block
  (d1(k)'s consumer is a block away and yields its slot); small first
  blocks fill the pipe fast and two 512-col drain blocks shorten the
  final serial tail. PSUM: a1 double-buffered, and a
  double-buffered "mid" arena reused by a2 -> v1 -> po (their lifetimes
  are sequential). x slabs are buffered deep enough (5) that input DMAs
  never wait on the residual read of x two blocks earlier (this WAR was
  a 12us/2-block serializer). Input DMAs ride Sync HWDGE, const loads
  ride Scalar/GpSimd queues, outputs ride Sync (fast teardown quiesce).
"""

import sys

import numpy as np

if "/opt/trn_rl_repo" not in sys.path:
    sys.path.insert(0, "/opt/trn_rl_repo")

import concourse.bass as bass
import concourse.tile as tile
from concourse import mybir

AF = mybir.ActivationFunctionType
F32 = mybir.dt.float32
F32R = mybir.dt.float32r
BF16 = mybir.dt.bfloat16

N_CORES = 8
B, S, N, D = 32, 256, 16, 6
HIDDEN = 32
BLADES = 32
DT = 0.01

TOK_TOTAL = B * S * N          # 131072 positions
TOK_CORE = TOK_TOTAL // N_CORES  # 16384
TPC = 4                        # tokens packed per 128-partition column
GROUPS = TOK_CORE // TPC       # 4096 columns per core
MM = 512                       # matmul free-dim (1 PSUM bank fp32)
BD = 1024                      # max block free-dim for elementwise ops
# column widths per block: big blocks for low per-op overhead, two small
# drain blocks so the final serial back-half chain is short
BLOCKS = [(0, 512), (512, 512), (1024, 1024), (2048, 1024), (3072, 512), (3584, 512)]

KP = TPC * D                   # 24 partitions for out
KPI = KP + 1                   # + constant ones row carrying the c1 bias


def _build_nc():
    nc = bass.Bass()

    xg = nc.dram_tensor("xg", [KPI, GROUPS], F32R, kind="ExternalInput")
    l12 = nc.dram_tensor("l12", [KPI, 256], F32R, kind="ExternalInput")
    l2w = nc.dram_tensor("l2w", [128, 128], BF16, kind="ExternalInput")
    l3 = nc.dram_tensor("l3", [128, 128], BF16, kind="ExternalInput")
    l4 = nc.dram_tensor("l4", [128, KP], BF16, kind="ExternalInput")
    outg = nc.dram_tensor("outg", [KP, GROUPS], F32, kind="ExternalOutput")

    with tile.TileContext(nc) as tc:
        with (
            tc.tile_pool(name="consts", bufs=1) as consts,
            tc.tile_pool(name="xin", bufs=5) as xin,
            tc.tile_pool(name="work", bufs=3) as work,
            # PSUM (8 banks total): a1 double-buffered [2 banks each], and a
            # double-buffered "mid" arena [2 banks each] through which a2,
            # v1, and po rotate (their lifetimes are strictly sequential
            # within a block: l2->d2, then l3->g1, then l4->res).
            tc.tile_pool(name="psA", bufs=2, space="PSUM") as psA,
            tc.tile_pool(name="psB", bufs=2, space="PSUM") as psB,
        ):
            # Const loads on the Scalar/GpSimd DGE queues; the Sync queue is
            # reserved for the x slabs so block 0's input lands first.
            sb_l12 = consts.tile([KPI, 256], F32R)
            nc.scalar.dma_start(out=sb_l12[:], in_=l12[:])
            sb_l1 = sb_l12[:, 0:128]
            sb_l2x = sb_l12[:, 128:256]
            sb_l2w = consts.tile([128, 128], BF16)
            nc.gpsimd.dma_start(out=sb_l2w[:], in_=l2w[:])
            sb_l3 = consts.tile([128, 128], BF16)
            nc.gpsimd.dma_start(out=sb_l3[:], in_=l3[:])
            sb_l4 = consts.tile([128, KP], BF16)
            nc.gpsimd.dma_start(out=sb_l4[:], in_=l4[:])

            # Dummy first activation: walrus attaches the ACT table load to
            # the first Activation instruction, which can then carry only a
            # single sync wait. Give it a single-wait warm-up op.
            warm = consts.tile([1, 128], F32)
            nc.vector.memset(warm[:], 0.0)
            nc.scalar.activation(warm[:], warm[:], AF.Derivative_silu)

            # HAM keep-warm: the PE drops to 1.2 GHz unless a ~3.4us window
            # stays busy. Scratch matmuls (zeros -> overwritten targets)
            # warm it during the initial DMA wait and bridge the per-block
            # matmul-free lulls so real matmuls stream at 2.4 GHz.
            scr = consts.tile([128, MM], BF16)
            nc.vector.memset(scr[:], 0.0)
            wu = psA.tile([128, BD], F32, tag="a1")
            for _ in range(8):
                nc.tensor.matmul(wu[:, :MM], scr[:, :128], scr[:],
                                 start=True, stop=True)



            def stage_b_early(st):
                """d2 -> v1 for the previous block. Issued right after
                tau1(k) so d2(k-1) — whose input has been ready since last
                block — takes the second ACT slot instead of queueing behind
                d1(k); the whole back-half chain shifts ~1 ACT op earlier."""
                c0, wd, mid, d1, sb_x = st
                ws = slice(0, wd)
                d2 = work.tile([128, wd], BF16, tag=f"d2_{wd}")
                nc.scalar.activation(d2[:], mid[:, ws], AF.Derivative_silu)

                # v1 = blockdiag(diag(w3) W2)^T @ d2, overwriting a2
                # (consumed by d2 above)
                for h in range(wd // MM):
                    ms = bass.ts(h, MM)
                    nc.tensor.matmul(mid[:, ms], sb_l3[:], d2[:, ms],
                                     start=True, stop=True)

            def stage_b(st):
                """g1 -> po -> out for the previous block (after the w(k)
                DVE op so w leads the DVE queue). v1/po reuse a2's banks."""
                c0, wd, mid, d1, sb_x = st
                ws = slice(0, wd)
                # g1 = v1 * d1
                g1 = work.tile([128, wd], BF16, tag=f"g1_{wd}")
                nc.vector.tensor_mul(g1[:], mid[:, ws], d1[:, ws])

                # po = blockdiag(Bout) @ g1, overwriting v1
                for h in range(wd // MM):
                    ms = bass.ts(h, MM)
                    nc.tensor.matmul(mid[:KP, ms], sb_l4[:], g1[:, ms],
                                     start=True, stop=True)

                # out = x + po  (PSUM + SBUF -> SBUF, then DMA out on the
                # Sync HWDGE queue — idle after the inputs, and it quiesces
                # much faster at NEFF teardown than the GpSimd SWDGE path)
                sb_o = work.tile([KP, wd], F32, tag=f"o_{wd}")
                nc.vector.tensor_add(sb_o[:], mid[:KP, ws],
                                     sb_x[:KP, ws].bitcast(F32))
                nc.sync.dma_start(out=outg[:, c0 : c0 + wd], in_=sb_o[:])

            pending = None
            prev_a1 = None
            for c0, wd in BLOCKS:
                # PSUM tiles stay BD-wide (fixed bank layout); the small
                # drain blocks just use the first wd columns.
                sb_x = xin.tile([KPI, wd], F32R, tag=f"x_{wd}")
                nc.sync.dma_start(out=sb_x[:], in_=xg[:, c0 : c0 + wd])

                # a1 = blockdiag(A) @ x + c1 (c1 rides the ones row)
                a1 = psA.tile([128, BD], F32, tag="a1")
                for h in range(wd // MM):
                    ms = bass.ts(h, MM)
                    nc.tensor.matmul(a1[:, ms], sb_l1, sb_x[:, ms],
                                     start=True, stop=True)

                # a2 = W2 silu(a1) + c2 split as (W2/2) a1 + (W2/2)(a1*tau1):
                # the first term composes with the input layer, so it streams
                # straight from x (l2x = blockdiag(W2 A / 2) with bias row
                # W2 c1 / 2 + c2).
                mid = psB.tile([128, BD], F32, tag="mid")
                for h in range(wd // MM):
                    ms = bass.ts(h, MM)
                    nc.tensor.matmul(mid[:, ms], sb_l2x, sb_x[:, ms],
                                     start=True, stop=False)

                # fillers into the previous a1 buffer (readers done; the
                # next block's l1 start=True reset clobbers them) bridge the
                # PE lull while ACT computes d2(k-1)
                if prev_a1 is not None:
                    for _ in range(2):
                        nc.tensor.matmul(prev_a1[:, :MM], scr[:, :128],
                                         scr[:], start=True, stop=True)
                prev_a1 = a1

                # tau = tanh(0.5*a1)
                tau = work.tile([128, wd], BF16, tag=f"tau_{wd}")
                nc.scalar.activation(tau[:], a1[:, :wd], AF.Tanh, scale=0.5)

                if pending is not None:
                    stage_b_early(pending)

                # d1 = silu'(a1) (consumer g1 is a block away, so it can
                # yield the ACT slot to d2(k-1) above)
                d1 = work.tile([128, wd], BF16, tag=f"d1_{wd}")
                nc.scalar.activation(d1[:], a1[:, :wd], AF.Derivative_silu)

                # w = a1 * tau1  (the only layer-1 elementwise product);
                # issued before stage_b so it leads the DVE queue.
                w = work.tile([128, wd], BF16, tag=f"w_{wd}")
                nc.vector.tensor_mul(w[:], a1[:, :wd], tau[:])

                # Previous block's back half: d2 follows d1(k) on ACT, its
                # matmuls precede the w-dependent l2w below on the PE queue.
                if pending is not None:
                    stage_b(pending)

                # a2 += blockdiag(W2/2) @ w
                for h in range(wd // MM):
                    ms = bass.ts(h, MM)
                    nc.tensor.matmul(mid[:, ms], sb_l2w[:], w[:, ms],
                                     start=False, stop=True)

                pending = (c0, wd, mid, d1, sb_x)

            stage_b(pending)

    return nc


def _split_multi_waits(nc):
    """This walrus build rejects engine instructions carrying more than one
    sync wait ("Too many sync wait commands"). Hoist all but one wait of
    each instruction onto standalone NoOps issued just before it on the
    same engine (engines execute their queue in order, so semantics are
    preserved)."""
    for f in nc.m.functions:
        for b in f.blocks:
            insts = list(b.instructions)
            out = []
            changed = False
            for inst in insts:
                # This walrus build also rejects the raw-ISA
                # EVENT_SEMAPHORE_RANGE_CLEAR Tile emits at context end
                # ("ISA wrong length" — ISA table version skew). The NEFF
                # preamble re-initializes semaphores, so drop it.
                if (
                    type(inst).__name__ == "InstISA"
                    and getattr(inst, "op_name", "") == "EVENT_SEMAPHORE_RANGE_CLEAR"
                ):
                    changed = True
                    continue
                si = getattr(inst, "sync_info", None)
                waits = list(si.on_wait) if si is not None and si.on_wait else []
                if len(waits) > 1:
                    changed = True
                    for k, w in enumerate(waits[:-1]):
                        nop = mybir.InstNoOp(name=f"{inst.name}-w{k}", ins=[], outs=[])
                        nop.engine = inst.engine
                        nop.sync_info = mybir.SyncInfo(on_wait=[w], on_update=[])
                        out.append(nop)
                    inst.sync_info = mybir.SyncInfo(
                        on_wait=[waits[-1]], on_update=list(si.on_update or [])
                    )
                out.append(inst)
            if changed:
                b.instructions = out
    return nc


_NC_CACHE = None


def _get_nc():
    global _NC_CACHE
    if _NC_CACHE is None:
        _NC_CACHE = _split_multi_waits(_build_nc())
    return _NC_CACHE


def _prep_weights(W_in, b_in, W1, b1, W2, b2, W3, b3):
    """Host-side constant folding into the kernel's stationary layouts."""
    import ml_dtypes

    W_in = np.asarray(W_in, np.float64)
    b_in = np.asarray(b_in, np.float64)
    W1 = np.asarray(W1, np.float64)
    b1 = np.asarray(b1, np.float64)
    W2 = np.asarray(W2, np.float64)
    b2 = np.asarray(b2, np.float64)
    W3 = np.asarray(W3, np.float64)

    Win0 = W_in[:, ::BLADES]            # [6, 8]
    bin0 = b_in[::BLADES]               # [8]
    A = W1 @ Win0.T                     # [32, 6]
    c1 = W1 @ bin0 + b1[:, 0]           # [32]
    c2 = b2[:, 0]                       # [32]
    w3 = W3[0, :]                       # [32]

    # Bout[d, c]: out[d] += dt*dx[d+3] (d<3), -dt*dx[d-3] (d>=3); dx = A^T g1
    Bout = np.zeros((D, HIDDEN))
    Bout[0:3, :] = DT * A[:, 3:6].T
    Bout[3:6, :] = -DT * A[:, 0:3].T

    # a2 = W2 silu(a1) + c2 = (W2/2) a1 + (W2/2)(a1 tau1) + c2, and
    # (W2/2) a1 = (W2 A / 2) x + W2 c1 / 2  composes with the input layer.
    A2x = 0.5 * W2 @ A                  # [32, 6]
    c2x = 0.5 * W2 @ c1 + c2            # [32]

    l12 = np.zeros((KPI, 256), np.float32)
    l1 = l12[:, 0:128]
    l2xm = l12[:, 128:256]
    l2wm = np.zeros((128, 128), ml_dtypes.bfloat16)
    l3 = np.zeros((128, 128), ml_dtypes.bfloat16)
    l4 = np.zeros((128, KP), ml_dtypes.bfloat16)
    for tl in range(TPC):
        # l1[6tl+d, 32tl+c] = A[c, d]; l1[24, 32tl+c] = c1[c]
        l1[6 * tl : 6 * tl + 6, 32 * tl : 32 * tl + 32] = A.T.astype(np.float32)
        l1[KP, 32 * tl : 32 * tl + 32] = c1.astype(np.float32)
        # l2x[6tl+d, 32tl+c] = A2x[c, d]; ones row carries c2x
        l2xm[6 * tl : 6 * tl + 6, 32 * tl : 32 * tl + 32] = A2x.T.astype(
            np.float32
        )
        l2xm[KP, 32 * tl : 32 * tl + 32] = c2x.astype(np.float32)
        # l2w[32tl+ci, 32tl+co] = W2[co, ci] / 2
        l2wm[32 * tl : 32 * tl + 32, 32 * tl : 32 * tl + 32] = (
            0.5 * W2.T
        ).astype(ml_dtypes.bfloat16)
        # l3[32tl+co, 32tl+ci] = w3[co] * W2[co, ci]
        l3[32 * tl : 32 * tl + 32, 32 * tl : 32 * tl + 32] = (
            w3[:, None] * W2
        ).astype(ml_dtypes.bfloat16)
        # l4[32tl+c, 6tl+d] = Bout[d, c]
        l4[32 * tl : 32 * tl + 32, 6 * tl : 6 * tl + 6] = Bout.T.astype(
            ml_dtypes.bfloat16
        )

    return {
        "l12": l12,
        "l2w": l2wm,
        "l3": l3,
        "l4": l4,
    }


def _shard_x(x):
    """[B,S,N,D] -> list of per-core [25, GROUPS] arrays (row 24 = 1.0)."""
    xf = np.ascontiguousarray(np.asarray(x, np.float32)).reshape(TOK_TOTAL, D)
    shards = []
    for c in range(N_CORES):
        xc = xf[c * TOK_CORE : (c + 1) * TOK_CORE]          # [16384, 6]
        xgc = np.empty((KPI, GROUPS), np.float32)
        xgc[:KP] = xc.reshape(GROUPS, TPC, D).transpose(1, 2, 0).reshape(KP, GROUPS)
        xgc[KP] = 1.0
        shards.append(xgc)
    return shards


def _unshard_out(outs):
    """list of per-core [24, GROUPS] -> [B,S,N,D]."""
    full = np.empty((TOK_TOTAL, D), np.float32)
    for c, og in enumerate(outs):
        oc = (
            np.asarray(og)
            .reshape(TPC, D, GROUPS)
            .transpose(2, 0, 1)
            .reshape(TOK_CORE, D)
        )
        full[c * TOK_CORE : (c + 1) * TOK_CORE] = oc
    return full.reshape(B, S, N, D)


# Test-harness knobs (ignored in normal use): set kernel._TRACE = True to
# collect an NTFF profile; the BassKernelResults lands in kernel._LAST_RES.
_TRACE = False
_LAST_RES = None


def kernel(x, W_in, b_in, W1, b1, W2, b2, W3, b3):
    global _LAST_RES
    from concourse.bass_utils import run_bass_kernel_spmd

    nc = _get_nc()
    consts = _prep_weights(W_in, b_in, W1, b1, W2, b2, W3, b3)
    shards = _shard_x(x)
    in_maps = [{"xg": shards[c], **consts} for c in range(N_CORES)]
    res = run_bass_kernel_spmd(nc, in_maps, list(range(N_CORES)), trace=_TRACE)
    _LAST_RES = res
    return _unshard_out([res.results[c]["outg"] for c in range(N_CORES)])
